# revision 17
# baseline (speedup 1.0000x reference)
"""AttnBlock (GroupNorm + cross-attention + proj + residual) on 8 trn2 cores.

Problem (hardcoded shapes): x, c: [2, 128, 16, 16, 16] fp32; C=128 channels,
N=4096 spatial tokens, 4 groups of 32 channels.

  h  = GN(x; g1, b1)            c_ = GN(c; g2, b2)
  q = wq c_ + bq ; k = wk h + bk ; v = wv h + bv
  S[b,i,j] = <q[:,i], k[:,j]> / sqrt(C) ;  A = softmax_j(S)
  out = x + wp (v A^T) + bp

Folded formulation (x-side GroupNorm folded through the attention algebra,
so attention runs directly on RAW bf16 x):
  With D = diag(A1), hn = A1*x + B1 (per-channel affine from GN stats):
    scores  S'[j,i] = x[:,j]^T q'[:,i],  q' = D Wk^T q  (the per-i constant
            beta_k^T q drops out of softmax_j)
    q = Wq cn + bq  =>  q' = D (G cn + g0),  G = Wk^T Wq, g0 = Wk^T bq
    U = X P  (raw x as values);  out = x + M D (U/d) + beta_f,
            M = Wp Wv,  beta_f = M B1 + Wp bv + bp
  Host packs G^T, M^T and the group-mean projector Gproj = gavg @ bc4.

Sharding: 8 cores, core m -> batch b=m//4, query rows i0=(m%4)*1024 .. +1024.
Inputs arrive column-rotated so the core's query/residual slice is cols 0:I.

Schedule: all input DMAs issue up-front (c halves + x halves split across the
two HW-DGE queues; wall/vall + xt on the gpsimd SW-DGE, xt gated behind x).
GroupNorm stats chase the DMA on DVE (bn_stats for both x and c); a PE warm
burst of junk fp32 matmuls gated on wall keeps the HAM clock at 8/8 from
~11us so the whole attention loop runs at 2.4GHz.  The loop is a software-
pipelined scores->exp->AV chain (ACT exp is the wall); denominators
accumulate via 4 col-group ones-matmuls (tile_position) into PSUM rows
{0,32,64,96}.  Tail: batched [2,512] dsum/Ln/Exp, K=2 broadcast matmuls,
DVE combine, y halves DMA'd on both HW queues.
"""

import ml_dtypes
import numpy as np

import concourse.bass as bass
import concourse.tile as tile
from concourse import mybir
from concourse.bass_utils import run_bass_kernel_spmd

N_CORES = 8
C = 128
N = 4096          # tokens per batch
I = 1024          # query rows per core
NG = 4            # groups
EPS = 1e-6
SCALE = 1.0 / np.sqrt(C)
JB = N // 128     # 32 j-blocks
F32 = mybir.dt.float32
F32R = mybir.dt.float32r
BF16 = mybir.dt.bfloat16

MM_DT = BF16
ATTN_DT = BF16

DEBUG_OUTS = False


class SlimTC(tile.TileContext):
    """TileContext with a slimmer kernel-tail: one all-engine barrier instead
    of two.  The second barrier only orders other engines behind the sem
    clears; each engine's own stream still completes before halt, and NRT
    doesn't restart streams until all engines halt, so reruns stay safe."""

    def _drain_and_barrier(self, tick_clock, wait_clock):
        from concourse.vector_clock import ScopedClock
        drain_inst = self.nc.sync.drain()
        wait_clock.add_sem_waits(
            drain_inst.ins, ScopedClock({None: tick_clock.global_clock})
        )
        done = self.nc.alloc_semaphore("tail_done")
        drain_inst.then_inc(done)
        self.nc.gpsimd.wait_ge(done, 1)
        assert self.sems is not None
        popped = self.nc._tile_sem_poison_stack.pop()
        assert popped is self._sem_poison
        sems = list(self.sems.allocated().values()) + [done]
        self.nc.clear_and_free_semaphores(sems)


def cap_sync_waits(nc):
    """Split multi-wait instructions: the pinned walrus accepts at most one
    sync wait per instruction ("Too many sync wait commands"). Hoist extra
    waits into single-wait NOPs inserted just before, on the same engine."""
    ctr = 0
    for f in nc.m.functions:
        for b in f.blocks:
            out = []
            for inst in b.instructions:
                si = inst.sync_info
                if si is not None and si.on_wait and len(si.on_wait) > 1:
                    waits = list(si.on_wait)
                    for w in waits[:-1]:
                        ctr += 1
                        out.append(mybir.InstNoOp(
                            name=f"I-waitsplit-{ctr}",
                            engine=inst.engine,
                            bass_nofuse=True,
                            sync_info=mybir.SyncInfo(on_wait=[w], on_update=[]),
                        ))
                    si.on_wait = waits[-1:]
                out.append(inst)
            b.instructions = out


def build_program():
    nc = bass.Bass("TRN2", target_bir_lowering=False, debug=False)

    # I/O.  xb/cb arrive ROTATED per core (columns rolled by -i0) so the
    # query/residual slice is always columns 0:1024; attention is
    # permutation-invariant in j, so the rotated frame is safe.
    xb = nc.declare_dram_parameter("xb", [C, N], BF16, isOutput=False)
    cb = nc.declare_dram_parameter("cb", [C, N], BF16, isOutput=False)
    # packed weights [C, 3C]: Gt = (Wk^T Wq)^T | Mt = (Wp Wv)^T | Gproj
    wall = nc.declare_dram_parameter("wall", [C, 3 * C], F32, isOutput=False)
    # packed per-channel vectors [C, 7]: 0:g0 1:t0 2:g1 3:b1 4:g2 5:b2 6:ones
    vall = nc.declare_dram_parameter("vall", [C, 7], F32, isOutput=False)
    # host-transposed x blocks: xtb[j, jb*C + c] = x[c, jb*128 + j]
    xtb = nc.declare_dram_parameter("xtb", [C, JB * C], BF16, isOutput=False)
    y = nc.declare_dram_parameter("y", [C, I], F32, isOutput=True)

    HN = N // 2

    with SlimTC(nc) as tc:
        with (
            tc.tile_pool(name="persist", bufs=1) as per,
            tc.tile_pool(name="smalls", bufs=1) as sm,
            tc.tile_pool(name="ptiles", bufs=6) as pp,
        ):
            eps128_t = sm.tile([C, 1], F32, tag="eps128")
            nc.vector.memset(eps128_t[:], EPS)
            zero128_t = sm.tile([C, 1], F32, tag="zero128")
            nc.vector.memset(zero128_t[:], 0.0)
            zero1_t = sm.tile([2, 1], F32, tag="zero1")
            nc.vector.memset(zero1_t[:], 0.0)
            # warm the ACT table set (Ln+Exp) off the critical path
            warm_t = sm.tile([1, 1], F32, tag="warm")
            nc.vector.memset(warm_t[:], 1.0)
            # K=1 all-ones lhsT rows for the reciprocal broadcast matmuls
            # (rows 0 and 32: matmul lhsT/rhs must share a base partition
            # in {0, 32, 64}, so the two d-rows live at partitions 0/32)
            ones1_t = sm.tile([33, C], F32, tag="ones1")
            nc.vector.memset(ones1_t[:], 1.0)
            ones1_r = sm.tile([33, C], F32R, tag="ones1_r")
            nc.vector.tensor_copy(ones1_r[:], ones1_t[:])
            # tail d-row pair-sums land in rows {0, 32}; zero the rest so
            # the batched Ln over the contiguous [33, 512] range reads
            # initialized data (lanes 1..31 are dead weight, cost nothing)
            dsum = sm.tile([33, 512], F32, tag="dsum")
            nc.vector.memset(dsum[:], 1.0)

            # ---- input DMA: everything up-front ----
            # wall/vall on the gpsimd SW-DGE (land ~8us, gate the PE warm
            # burst); c halves + x halves split across the two HW-DGE queues
            # so both land as early as possible; xt follows on the gpsimd
            # queue gated behind x (it isn't consumed until the AV loop).
            wall_t = per.tile([C, 3 * C], F32, tag="wall")
            vall_t = sm.tile([C, 7], F32, tag="vall")
            nc.gpsimd.dma_start(wall_t[:], wall[:])
            nc.gpsimd.dma_start(vall_t[:], vall[:])

            x_t = per.tile([C, N], BF16, tag="x")
            c_t = per.tile([C, N], BF16, tag="c")
            xt_t = per.tile([C, JB, C], BF16, tag="xt")
            f_t = per.tile([C, I], F32, tag="f")   # tail result

            # c first on both HW queues (it gates the longest chain), then x
            # behind it on the same queues (per-queue FIFO sequences the
            # transfers); x's tail quarters are split finer so the last
            # bn_stats chunks start as early as possible.  Aggregate DDR is
            # ~200GB/s with all 8 cores pulling, so ordering is everything.
            nc.sync.dma_start(c_t[:, 0:HN], cb[:, 0:HN])
            nc.scalar.dma_start(c_t[:, HN:N], cb[:, HN:N])
            nc.sync.dma_start(x_t[:, 0:HN], xb[:, 0:HN])
            nc.scalar.dma_start(x_t[:, HN:HN + 1024], xb[:, HN:HN + 1024])
            nc.sync.dma_start(
                x_t[:, HN + 1024:HN + 1536], xb[:, HN + 1024:HN + 1536])
            nc.scalar.dma_start(
                x_t[:, HN + 1536:N], xb[:, HN + 1536:N])

            # xt behind x: a dummy gpsimd write INTO xt_t creates a WAW dep
            # the scheduler cannot reorder around (a pure read gate gets
            # hoisted), so the xt transfer only enters the ring after x has
            # landed.  xt block jb isn't consumed until AV iteration jb, so
            # it streams with plenty of slack.
            nc.gpsimd.tensor_copy(
                xt_t[:, 0, 0:1], x_t[:, N - 1:N])
            nc.gpsimd.tensor_copy(
                xt_t[:, 16, 0:1], x_t[:, N - 1:N])
            for hf in range(2):
                hsl = slice(hf * 16, (hf + 1) * 16)
                nc.gpsimd.dma_start(
                    xt_t[:, hsl, :], xtb[:, hf * 16 * C:(hf + 1) * 16 * C]
                )

            gt_t = wall_t[:, 0 * C:1 * C]
            mt_t = wall_t[:, 1 * C:2 * C]
            gproj_t = wall_t[:, 2 * C:3 * C]
            g0_t = vall_t[:, 0:1]
            t0_t = vall_t[:, 1:2]
            g1_t = vall_t[:, 2:3]
            b1_t = vall_t[:, 3:4]
            g2_t = vall_t[:, 4:5]
            b2_t = vall_t[:, 5:6]
            ones_t = vall_t[:, 6:7]

            # ACT table-set warm (Ln+Exp), after the DMA triggers on ACT
            nc.scalar.activation(out=warm_t[:], in_=warm_t[:],
                                 func=mybir.ActivationFunctionType.Ln)
            nc.scalar.activation(out=warm_t[:], in_=warm_t[:],
                                 func=mybir.ActivationFunctionType.Exp)

            # rounded/typed copies for the PE
            mt_r = per.tile([C, C], MM_DT, tag="mt_r")
            nc.vector.tensor_copy(mt_r[:], mt_t[:])
            ones_a = sm.tile([C, 1], ATTN_DT, tag="ones_a")
            nc.vector.tensor_copy(ones_a[:], ones_t[:])

            # ---- stats on DVE, chasing the DMA halves ----
            # Emission order matters: the DVE is strict FIFO, so the whole
            # c-side chain is emitted BEFORE the x bn_stats -- otherwise the
            # c ops queue behind x chunks that are still waiting on DMA.
            CH = 512
            stats_x = sm.tile([C, 8, 6], F32, tag="stats_x")
            stats_c = sm.tile([C, 8, 6], F32, tag="stats_c")
            for ch in range(8):
                csl = slice(ch * CH, (ch + 1) * CH)
                nc.vector.bn_stats(out=stats_c[:, ch, :], in_=c_t[:, csl])

            # ---- group-norm channel affine A[c], B[c] ----
            # d2 = [mean_c, E[x^2]_c]; one Gproj matmul aggregates+broadcasts
            # group means; rstd = exp(-0.5*ln(var+eps)).
            def d2_from_stats(stats, label):
                # bn_aggr -> [mean, var] -> [mean, mean^2+var] in one fused
                # in-place op: mv1 = (mv0 * mv0) + mv1
                mv = sm.tile([C, 2], F32, tag=f"mv_{label}")
                nc.vector.bn_aggr(out=mv[:], in_=stats[:])
                nc.vector.scalar_tensor_tensor(
                    out=mv[:, 1:2], in0=mv[:, 0:1], scalar=mv[:, 0:1],
                    in1=mv[:, 1:2],
                    op0=mybir.AluOpType.mult, op1=mybir.AluOpType.add,
                )
                return mv

            def gn_affine_d2(gnps, d2, label):
                cps = gnps.tile([C, 2], F32, tag=f"ch_{label}")
                nc.tensor.matmul(cps[:], gproj_t[:], d2[:], start=True, stop=True)
                csb = sm.tile([C, 2], F32, tag=f"csb_{label}")
                nc.vector.tensor_copy(csb[:], cps[:])
                var = sm.tile([C, 1], F32, tag=f"var_{label}")
                nc.vector.tensor_mul(var[:], csb[:, 0:1], csb[:, 0:1])
                nc.vector.tensor_sub(var[:], csb[:, 1:2], var[:])
                return csb, var

            def gn_affine_fin(csb, var, gamma_t, beta_t, label):
                lnv = sm.tile([C, 1], F32, tag=f"lnv_{label}")
                nc.scalar.activation(
                    out=lnv[:], in_=var[:], func=mybir.ActivationFunctionType.Ln,
                    bias=eps128_t[:], scale=1.0,
                )
                rstd = sm.tile([C, 1], F32, tag=f"rstd_{label}")
                nc.scalar.activation(
                    out=rstd[:], in_=lnv[:], func=mybir.ActivationFunctionType.Exp,
                    scale=-0.5,
                )
                a_t = sm.tile([C, 1], F32, tag=f"A_{label}")
                nc.vector.tensor_mul(a_t[:], rstd[:], gamma_t[:])
                b_t = sm.tile([C, 1], F32, tag=f"B_{label}")
                nc.vector.tensor_mul(b_t[:], csb[:, 0:1], a_t[:])
                nc.vector.tensor_sub(b_t[:], beta_t[:], b_t[:])
                return a_t, b_t

            q_t = per.tile([C, I], ATTN_DT, tag="q")
            bf_t = sm.tile([C, 1], F32, tag="beta_f")
            s2_t = sm.tile([C, 1], F32, tag="s2")

            with (
                tc.tile_pool(
                    name="gn_ps", bufs=1, space=bass.MemorySpace.PSUM
                ) as gnps,
                tc.tile_pool(
                    name="proj_ps", bufs=1, space=bass.MemorySpace.PSUM
                ) as pps,
            ):
                # PE warm burst FIRST in the PE queue: 3 junk fp32 matmuls
                # on wall data (lands ~8us) keep the PE busy ~3.5us so the
                # HAM clock reaches 8/8 before any real matmul, and the gaps
                # to the gn/q'/scores matmuls stay under the ~3.4us MID
                # window -- the whole attention loop then runs at 2.4GHz.
                junk_ps = gnps.tile([C, 384], F32, tag="junk")
                for w in range(4):
                    nc.tensor.matmul(
                        junk_ps[:], wall_t[:, 0:C], wall_t[:, 0:384],
                        start=True, stop=True,
                    )

                # c path (fully emitted before any x-side DVE work)
                d2c = d2_from_stats(stats_c, "c")
                cpc, varc = gn_affine_d2(gnps, d2c, "c")
                ac_t, bc_t = gn_affine_fin(cpc, varc, g2_t, b2_t, "c")

                # q' = A1 * (G (A2*c + B2) + g0) on RAW c:
                # fold A2 into the weight rows (Gt' = A2 . Gt) and B2 into
                # the bias (s2 = A1*(G B2 + g0)); no cn tensor at all
                gtp_r = per.tile([C, C], MM_DT, tag="gtp_r")
                nc.vector.tensor_scalar(
                    out=gtp_r[:], in0=gt_t[:], scalar1=ac_t[:],
                    scalar2=zero128_t[:],
                    op0=mybir.AluOpType.mult, op1=mybir.AluOpType.add,
                )
                gb2_ps = gnps.tile([C, 1], F32, tag="gb2")
                nc.tensor.matmul(gb2_ps[:], gt_t[:], bc_t[:],
                                 start=True, stop=True)

                # q'' matmuls right after gb2 in the PE queue -- they only
                # need the folded weights, NOT the x stats, so they must not
                # queue behind the x-side cpx matmul
                qps = pps.tile([C, I], F32, tag="q")
                for ih in range(2):
                    sl = slice(ih * 512, (ih + 1) * 512)
                    nc.tensor.matmul(
                        qps[:, sl], gtp_r[:], c_t[:, sl],
                        start=True, stop=True,
                    )
                # two more junk matmuls bridge the PE-idle window between
                # q'' and the x-side matmuls (keeps HAM at 8/8)
                for w in range(2):
                    nc.tensor.matmul(
                        junk_ps[:], wall_t[:, 0:C], wall_t[:, 0:384],
                        start=True, stop=True,
                    )

                # x stats + path (x lands after c; its DVE ops come last)
                for ch in range(8):
                    csl = slice(ch * CH, (ch + 1) * CH)
                    nc.vector.bn_stats(out=stats_x[:, ch, :], in_=x_t[:, csl])
                d2x = d2_from_stats(stats_x, "x")
                cpx, varx = gn_affine_d2(gnps, d2x, "x")
                ax_t, bx_t = gn_affine_fin(cpx, varx, g1_t, b1_t, "x")

                nc.vector.scalar_tensor_tensor(
                    out=s2_t[:], in0=gb2_ps[:], scalar=g0_t[:], in1=ax_t[:],
                    op0=mybir.AluOpType.add, op1=mybir.AluOpType.mult,
                )

                # beta_f = M @ B1 + t0 (tail-only; after q' in the PE queue)
                bf_ps = pps.tile([C, 1], F32, tag="bf")
                nc.tensor.matmul(bf_ps[:], mt_t[:], bx_t[:],
                                 start=True, stop=True)
                nc.vector.tensor_add(bf_t[:], bf_ps[:], t0_t[:])
                # q' evac on ACT as ONE [C, I] op (two halves would
                # serialize on ACT anyway, paying the init twice)
                nc.scalar.activation(
                    out=q_t[:], in_=qps[:],
                    func=mybir.ActivationFunctionType.Identity,
                    bias=s2_t[:], scale=ax_t[:],
                )

            # ---- attention ----
            # Software-pipelined: scores for jb+2 are emitted ahead of the
            # exp-dependent AV/denom work for jb, so the PE never stalls on
            # the ScalarE exp.  Denominators accumulate into 4 col-group
            # accumulators (tile_position) landing in PSUM rows {0,32,64,96}.
            o_sb = per.tile([C, I], MM_DT, tag="osb")
            zz_t = per.tile([C, I], F32, tag="zz")
            rb_sb = per.tile([C, I], F32, tag="rbsb")

            st_tiles = {}
            p_tiles = {}

            with tc.tile_pool(
                name="acc_ps", bufs=1, space=bass.MemorySpace.PSUM
            ) as acc:
                o_ps = acc.tile([C, I], F32, tag="o")
                d4_ps = acc.tile([C, 512], F32, tag="d4")
                nc.vector.memset(d4_ps[:], 0.0)

                with tc.tile_pool(
                    name="st_ps", bufs=2, space=bass.MemorySpace.PSUM
                ) as stp:
                    def emit_scores(jb):
                        st = stp.tile([C, I], F32, tag="st")
                        st_tiles[jb] = st
                        for ih in range(2):
                            nc.tensor.matmul(
                                st[:, ih * 512:(ih + 1) * 512],
                                x_t[:, jb * 128:(jb + 1) * 128],
                                q_t[:, ih * 512:(ih + 1) * 512],
                                start=True, stop=True,
                            )

                    def emit_exp(jb):
                        p_t = pp.tile([C, I], ATTN_DT, tag="p")
                        p_tiles[jb] = p_t
                        nc.scalar.activation(
                            out=p_t[:], in_=st_tiles.pop(jb)[:],
                            func=mybir.ActivationFunctionType.Exp,
                            scale=float(SCALE),
                        )

                    emit_scores(0)
                    emit_scores(1)
                    emit_exp(0)
                    for jb in range(JB):
                        if jb + 2 < JB:
                            emit_scores(jb + 2)
                        if jb + 1 < JB:
                            emit_exp(jb + 1)
                        p_t = p_tiles[jb]
                        first, last = jb == 0, jb == JB - 1
                        for ih in range(2):
                            sl = slice(ih * 512, (ih + 1) * 512)
                            nc.tensor.matmul(
                                o_ps[:, sl], xt_t[:, jb, :], p_t[:, sl],
                                start=first, stop=last,
                            )
                        if jb % 2 == 1:
                            for g in range(4):
                                jj, ih = jb - 1 + g // 2, g % 2
                                sl = slice(ih * 512, (ih + 1) * 512)
                                nc.tensor.matmul(
                                    d4_ps[32 * g:32 * g + 1, 0:512],
                                    ones_a[:], p_tiles[jj][:, sl],
                                    start=jb == 1, stop=last,
                                    tile_position=(0, 32 * g),
                                )
                            p_tiles.pop(jb - 1)
                            p_tiles.pop(jb)

                # denominator: one wide copy out of PSUM, pair-sum the group
                # rows into ONE [2,512] tile (0+64 -> d[0:512] in row 0,
                # 32+96 -> d[512:1024] in row 1), then a single batched
                # Ln+Exp(-1) reciprocal on the post-loop-idle ACT
                # (pair-add mixes a PSUM and an SBUF operand: two SBUF
                # operands must share a base partition, mixed ones need not)
                d4s = per.tile([C, 512], F32, tag="d4s")
                nc.vector.tensor_copy(d4s[:], d4_ps[:])
                for ih in range(2):
                    nc.vector.tensor_add(
                        dsum[32 * ih:32 * ih + 1, :],
                        d4_ps[32 * ih:32 * ih + 1, 0:512],
                        d4s[64 + 32 * ih:65 + 32 * ih, :],
                    )
                # one batched Ln+Exp over both d-rows (partitions 0 and 32;
                # the 31 untouched lanes in between cost nothing -- ACT time
                # is free-dim bound -- and their outputs are never read)
                lnd = sm.tile([33, 512], F32, tag="lnd")
                nc.scalar.activation(
                    out=lnd[:], in_=dsum[:],
                    func=mybir.ActivationFunctionType.Ln,
                )
                rsb2 = sm.tile([33, 512], F32R, tag="rsb2")
                nc.scalar.activation(
                    out=rsb2[:], in_=lnd[:],
                    func=mybir.ActivationFunctionType.Exp,
                    scale=-1.0,
                )

                # U out of PSUM with the A1 row-scale folded in, on the
                # post-loop-idle ACT (keeps DVE free for the d-path);
                # 2 halves so the z matmul starts after the first
                for ih in range(2):
                    sl = slice(ih * 512, (ih + 1) * 512)
                    nc.scalar.activation(
                        out=o_sb[:, sl], in_=o_ps[:, sl],
                        func=mybir.ActivationFunctionType.Identity,
                        scale=ax_t[:],
                    )
            with tc.tile_pool(
                name="tail_ps", bufs=1, space=bass.MemorySpace.PSUM
            ) as tlp:
                z_ps = tlp.tile([C, I], F32, tag="z")
                rb_ps = tlp.tile([C, I], F32, tag="rb")
                for ih in range(2):
                    sl = slice(ih * 512, (ih + 1) * 512)
                    nc.tensor.matmul(z_ps[:, sl], mt_r[:], o_sb[:, sl],
                                     start=True, stop=True)
                # junk matmul gated on the d-chain Ln keeps the PE busy
                # through the reciprocal latency so the rb matmuls run warm
                junk3_ps = tlp.tile([C, 512], F32, tag="junk3")
                nc.tensor.matmul(junk3_ps[:], wall_t[0:33, 0:C], lnd[:],
                                 start=True, stop=True)
                for ih in range(2):
                    sl = slice(ih * 512, (ih + 1) * 512)
                    nc.tensor.matmul(
                        rb_ps[:, sl], ones1_r[32 * ih:32 * ih + 1, :],
                        rsb2[32 * ih:32 * ih + 1, :],
                        start=True, stop=True,
                    )
                    # rb out of PSUM on DVE (ACT is busy with Ln/Exp);
                    # combine on DVE: f = (z * recip + beta_f) + x
                    nc.vector.tensor_copy(rb_sb[:, sl], rb_ps[:, sl])
                    nc.vector.tensor_tensor(
                        zz_t[:, sl], z_ps[:, sl], rb_sb[:, sl],
                        mybir.AluOpType.mult,
                    )
                    nc.vector.scalar_tensor_tensor(
                        out=f_t[:, sl], in0=zz_t[:, sl], scalar=bf_t[:],
                        in1=x_t[:, sl],
                        op0=mybir.AluOpType.add, op1=mybir.AluOpType.add,
                    )
                    if ih == 0:
                        nc.sync.dma_start(y[:, sl], f_t[:, sl])
                    else:
                        nc.scalar.dma_start(y[:, sl], f_t[:, sl])

    cap_sync_waits(nc)
    return nc


_PROGRAM = None


def _get_program():
    global _PROGRAM
    if _PROGRAM is None:
        _PROGRAM = build_program()
    return _PROGRAM


def _prep_in_maps(x, c, g1, b1, g2, b2, wq, bq, wk, bk, wv, bv, wp, bp):
    f = np.float32
    a = lambda v: np.asarray(v, f)
    ch = np.arange(C) // 32
    gproj = (ch[:, None] == ch[None, :]).astype(f) / 32.0
    gt = a(wq).T @ a(wk)             # lhsT for q'' = (Wk^T Wq) @ cn
    mt = (a(wp) @ a(wv)).T           # lhsT for z = (Wp Wv) @ Us
    wall = np.concatenate([gt, mt, gproj], axis=1)
    g0 = a(wk).T @ a(bq)
    t0 = a(wp) @ a(bv) + a(bp)
    vall = np.stack([
        g0, t0, a(g1), a(b1), a(g2), a(b2), np.ones(C, f),
    ], axis=1)                       # [C, 7]
    common = {
        "wall": np.ascontiguousarray(wall),
        "vall": np.ascontiguousarray(vall),
    }
    xf = a(x).reshape(2, C, N)
    cf = a(c).reshape(2, C, N)
    in_maps = []
    for m in range(N_CORES):
        b, quarter = m // 4, m % 4
        i0 = quarter * I
        # roll columns so this core's query/residual rows are columns 0:I;
        # attention is permutation-invariant in j so the rotated frame is safe
        xr = np.ascontiguousarray(
            np.roll(xf[b], -i0, axis=1)).astype(ml_dtypes.bfloat16)
        # xt[j, jb, c] = x[c, jb*128 + j]
        xt = np.ascontiguousarray(
            xr.reshape(C, JB, 128).transpose(2, 1, 0)).reshape(C, JB * C)
        in_maps.append({
            "xb": xr,
            "xtb": xt,
            "cb": np.ascontiguousarray(
                np.roll(cf[b], -i0, axis=1)).astype(ml_dtypes.bfloat16),
            **common,
        })
    return in_maps


def run_spmd(inputs, trace=False, **kw):
    nc = _get_program()
    in_maps = _prep_in_maps(**inputs)
    return run_bass_kernel_spmd(nc, in_maps, list(range(N_CORES)), trace=trace, **kw)


def kernel(**inputs) -> np.ndarray:
    res = run_spmd(inputs, trace=False)
    out = np.empty((2, C, N), np.float32)
    for m in range(N_CORES):
        b, quarter = m // 4, m % 4
        out[b][:, quarter * I:(quarter + 1) * I] = res.results[m]["y"]
    return out.reshape(2, C, 16, 16, 16)
